# revision 1
# baseline (speedup 1.0000x reference)
"""Trainium2 Bass kernel for nn_MemLayer (retrieval_knn).

Math:  out[b,o] = -mean_d (x[b,d] - w[o,d])^2 + bias[o]
              =  (2/D) * (x @ w.T)[b,o]  -  ||x_b||^2/D  +  (bias[o] - ||w_o||^2/D)

Strategy:
  - Data-parallel shard x along batch across 8 NeuronCores (1024 rows each),
    replicate weights. No cross-core communication; gather outputs on host.
  - Per core: bf16 GEMM [1024,1024] @ [1024,4096] on the PE array with fp32
    PSUM accumulation (K=1024 on partitions as 8 k-tiles of 128).
    The 2/D scale is folded into the bf16 weights on the host.
  - Corrections stay fp32 and are fused into PSUM eviction:
      * ACT engine:  out_sb = psum + xsq[p]         (per-partition bias)
      * DVE engine:  out_sb += v[o]                 (SBUF-resident broadcast row)
    where xsq = -||x_b||^2/D (fp32, host-computed from fp32 x) and
    v = bias - ||w_o||^2/D (fp32, host-computed from fp32 w).

Only the tiny rank-1 reductions (x_sq, w_sq) are computed on the host in fp32;
the O(B*O*D) GEMM runs on the PE arrays in bf16, which keeps the elementwise
relative error ~5e-6 (fp32-accumulated bf16 products of a D=1024 contraction).
"""

import numpy as np
import ml_dtypes

B, D, O = 8192, 1024, 4096
NCORES = 8
BL = B // NCORES     # 1024 rows per core
P = 128
KT = D // P          # 8 k-tiles
MT = BL // P         # 8 m-tiles
NTILE = 512          # one PSUM bank of fp32
NT = O // NTILE      # 8 n-tiles

_CACHE = {}


def _get_nc():
    if "nc" in _CACHE:
        return _CACHE["nc"]

    import concourse.bacc as bacc
    import concourse.tile as tile
    from concourse import mybir

    nc = bacc.Bacc("TRN2", target_bir_lowering=False)

    xk_d = nc.dram_tensor("xk", [P, KT, BL], mybir.dt.bfloat16, kind="ExternalInput")
    wk_d = nc.dram_tensor("wk", [P, KT, O], mybir.dt.bfloat16, kind="ExternalInput")
    xsq_d = nc.dram_tensor("xsq", [P, MT], mybir.dt.float32, kind="ExternalInput")
    v_d = nc.dram_tensor("v", [1, O], mybir.dt.float32, kind="ExternalInput")
    out_d = nc.dram_tensor("out", [P, MT, O], mybir.dt.float32, kind="ExternalOutput")

    with tile.TileContext(nc) as tc:
        with (
            tc.tile_pool(name="const", bufs=1) as cpool,
            tc.tile_pool(name="psum", bufs=8, space="PSUM") as ppool,
            tc.tile_pool(name="outp", bufs=2) as opool,
        ):
            xk_sb = cpool.tile([P, KT, BL], mybir.dt.bfloat16)
            nc.sync.dma_start(out=xk_sb[:], in_=xk_d[:])
            xsq_sb = cpool.tile([P, MT], mybir.dt.float32)
            nc.sync.dma_start(out=xsq_sb[:], in_=xsq_d[:])
            vb_sb = cpool.tile([P, O], mybir.dt.float32)
            nc.sync.dma_start(out=vb_sb[:], in_=v_d[:].to_broadcast([P, O]))
            # Load weights in k-chunks so the first matmuls don't wait on the
            # full 8MB transfer.
            wk_sb = cpool.tile([P, KT, O], mybir.dt.bfloat16)
            for kc in range(KT):
                nc.sync.dma_start(out=wk_sb[:, kc, :], in_=wk_d[:, kc, :])

            for mt in range(MT):
                out_sb = opool.tile([P, O], mybir.dt.float32)
                for nt in range(NT):
                    ps = ppool.tile([P, NTILE], mybir.dt.float32)
                    for kc in range(KT):
                        nc.tensor.matmul(
                            ps[:],
                            lhsT=xk_sb[:, kc, mt * P:(mt + 1) * P],
                            rhs=wk_sb[:, kc, nt * NTILE:(nt + 1) * NTILE],
                            start=(kc == 0),
                            stop=(kc == KT - 1),
                        )
                    ns = slice(nt * NTILE, (nt + 1) * NTILE)
                    nc.scalar.activation(
                        out_sb[:, ns],
                        ps[:],
                        mybir.ActivationFunctionType.Identity,
                        bias=xsq_sb[:, mt:mt + 1],
                    )
                    nc.vector.tensor_add(out_sb[:, ns], out_sb[:, ns], vb_sb[:, ns])
                nc.sync.dma_start(out=out_d[:, mt, :], in_=out_sb[:])

    nc.finalize()
    _CACHE["nc"] = nc
    return nc


def _prep_inputs(x, weights, bias):
    """Shard + lay out host inputs -> per-core in_maps."""
    bf16 = ml_dtypes.bfloat16
    x = np.asarray(x, dtype=np.float32)
    weights = np.asarray(weights, dtype=np.float32)
    bias = np.asarray(bias, dtype=np.float32)

    # [D, O] with 2/D folded in, k-tiled to [P, KT, O]
    wT = weights.T * np.float32(2.0 / D)
    wk = np.ascontiguousarray(
        wT.reshape(KT, P, O).transpose(1, 0, 2).astype(bf16)
    )
    w_sq = np.einsum("od,od->o", weights, weights)
    v = np.ascontiguousarray((bias - w_sq / np.float32(D)).reshape(1, O))

    in_maps = []
    for c in range(NCORES):
        xs = x[c * BL:(c + 1) * BL]                       # [BL, D] fp32
        xT = xs.T                                          # [D, BL]
        xk = np.ascontiguousarray(
            xT.reshape(KT, P, BL).transpose(1, 0, 2).astype(bf16)
        )
        xsq = -np.einsum("bd,bd->b", xs, xs) / np.float32(D)   # [BL]
        xsq_l = np.ascontiguousarray(xsq.reshape(MT, P).T)     # [P, MT]
        in_maps.append({"xk": xk, "wk": wk, "xsq": xsq_l, "v": v})
    return in_maps


def _gather(results):
    parts = []
    for c in range(NCORES):
        o = results[c]["out"]                              # [P, MT, O]
        parts.append(o.transpose(1, 0, 2).reshape(BL, O))
    return np.ascontiguousarray(np.concatenate(parts, axis=0))


def _run(in_maps, **kwargs):
    from concourse.bass_utils import run_bass_kernel_spmd

    nc = _get_nc()
    return run_bass_kernel_spmd(nc, in_maps, core_ids=list(range(NCORES)), **kwargs)


def kernel(x, weights, bias):
    in_maps = _prep_inputs(x, weights, bias)
    res = _run(in_maps)
    return _gather(res.results)


# revision 3
# speedup vs baseline: 1.0021x; 1.0021x over previous
"""Trainium2 Bass kernel for nn_MemLayer (retrieval_knn).

Math:  out[b,o] = -mean_d (x[b,d] - w[o,d])^2 + bias[o]
              =  (2/D) * (x @ w.T)[b,o]  -  ||x_b||^2/D  +  (bias[o] - ||w_o||^2/D)

Strategy:
  - Data-parallel shard x along batch across 8 NeuronCores (1024 rows each),
    replicate weights. No cross-core communication; gather outputs on host.
  - Per core: bf16 GEMM [1024,1024] @ [1024,4096] on the PE array with fp32
    PSUM accumulation. The 2/D scale is folded into the bf16 weights on host.
  - Schedule: n-tile outer; within an n-tile the contraction (kc) loop is
    OUTER across all 8 PSUM banks (one per m-tile), so the first matmuls only
    need one 256KB x-chunk + one 128KB w-chunk of DMA before the PE starts,
    and weight n-chunks stream in ~5x faster than the PE consumes them.
  - Corrections stay fp32, fused into PSUM eviction:
      * ACT:  out_sb = psum + xsq[p]   (per-partition bias, xsq = -||x||^2/D)
      * DVE:  out_sb += v[o]           (v = bias - ||w||^2/D, SBUF row bcast)
    then a 256KB DMA per (m,n) tile straight to DRAM (no output staging tail).

Only the tiny rank-1 reductions (x_sq, w_sq) are computed on the host in fp32;
the O(B*O*D) GEMM runs on the PE arrays in bf16 (elementwise output error
~1e-5 relative, fp32 accumulation).
"""

import numpy as np
import ml_dtypes

B, D, O = 8192, 1024, 4096
NCORES = 8
BL = B // NCORES     # 1024 rows per core
P = 128
KT = D // P          # 8 k-tiles
MT = BL // P         # 8 m-tiles
NTILE = 512          # one PSUM bank of fp32
NT = O // NTILE      # 8 n-tiles

_CACHE = {}


def _get_nc():
    if "nc" in _CACHE:
        return _CACHE["nc"]

    import concourse.bacc as bacc
    import concourse.tile as tile
    from concourse import mybir

    nc = bacc.Bacc("TRN2", target_bir_lowering=False)

    xk_d = nc.dram_tensor("xk", [P, KT, BL], mybir.dt.bfloat16, kind="ExternalInput")
    wk_d = nc.dram_tensor("wk", [NT, P, KT, NTILE], mybir.dt.bfloat16,
                          kind="ExternalInput")
    xsq_d = nc.dram_tensor("xsq", [P, MT], mybir.dt.float32, kind="ExternalInput")
    v_d = nc.dram_tensor("v", [1, O], mybir.dt.float32, kind="ExternalInput")
    out_d = nc.dram_tensor("out", [P, MT, O], mybir.dt.float32, kind="ExternalOutput")

    f32 = mybir.dt.float32
    bf16 = mybir.dt.bfloat16

    with tile.TileContext(nc) as tc:
        with (
            tc.tile_pool(name="const", bufs=1) as cpool,
            tc.tile_pool(name="psum", bufs=8, space="PSUM") as ppool,
            tc.tile_pool(name="outp", bufs=4) as opool,
        ):
            xk_sb = cpool.tile([P, KT, BL], bf16)
            wk_sb = cpool.tile([P, NT, KT, NTILE], bf16)
            xsq_sb = cpool.tile([P, MT], f32)
            vb_sb = cpool.tile([P, O], f32)

            # DMA issue order = priority. The PE's first matmul needs only
            # xk[:,0,:] + wk[0,:,0,:]; interleave the nt=0 k-chunks first.
            for kc in range(KT):
                nc.sync.dma_start(out=xk_sb[:, kc, :], in_=xk_d[:, kc, :])
                nc.sync.dma_start(out=wk_sb[:, 0, kc, :], in_=wk_d[0, :, kc, :])
            nc.sync.dma_start(out=xsq_sb[:], in_=xsq_d[:])
            nc.sync.dma_start(out=vb_sb[:], in_=v_d[:].to_broadcast([P, O]))
            for nt in range(1, NT):
                nc.sync.dma_start(out=wk_sb[:, nt, :, :], in_=wk_d[nt])

            for nt in range(NT):
                pss = []
                for mt in range(MT):
                    ps = ppool.tile([P, NTILE], f32, tag="ps")
                    pss.append(ps)
                for kc in range(KT):
                    for mt in range(MT):
                        nc.tensor.matmul(
                            pss[mt][:],
                            lhsT=xk_sb[:, kc, mt * P:(mt + 1) * P],
                            rhs=wk_sb[:, nt, kc, :],
                            start=(kc == 0),
                            stop=(kc == KT - 1),
                        )
                ns = slice(nt * NTILE, (nt + 1) * NTILE)
                for mt in range(MT):
                    ob = opool.tile([P, NTILE], f32)
                    nc.scalar.activation(
                        ob[:],
                        pss[mt][:],
                        mybir.ActivationFunctionType.Identity,
                        bias=xsq_sb[:, mt:mt + 1],
                    )
                    nc.vector.tensor_add(ob[:], ob[:], vb_sb[:, ns])
                    nc.sync.dma_start(out=out_d[:, mt, ns], in_=ob[:])

    nc.finalize()
    _CACHE["nc"] = nc
    return nc


def _prep_inputs(x, weights, bias):
    """Shard + lay out host inputs -> per-core in_maps."""
    bf16 = ml_dtypes.bfloat16
    x = np.asarray(x, dtype=np.float32)
    weights = np.asarray(weights, dtype=np.float32)
    bias = np.asarray(bias, dtype=np.float32)

    # [D, O] with 2/D folded in -> [NT, P, KT, NTILE]
    wT = weights.T * np.float32(2.0 / D)
    wk = np.ascontiguousarray(
        wT.reshape(KT, P, NT, NTILE).transpose(2, 1, 0, 3).astype(bf16)
    )
    w_sq = np.einsum("od,od->o", weights, weights)
    v = np.ascontiguousarray((bias - w_sq / np.float32(D)).reshape(1, O))

    in_maps = []
    for c in range(NCORES):
        xs = x[c * BL:(c + 1) * BL]                       # [BL, D] fp32
        xT = xs.T                                          # [D, BL]
        xk = np.ascontiguousarray(
            xT.reshape(KT, P, BL).transpose(1, 0, 2).astype(bf16)
        )
        xsq = -np.einsum("bd,bd->b", xs, xs) / np.float32(D)   # [BL]
        xsq_l = np.ascontiguousarray(xsq.reshape(MT, P).T)     # [P, MT]
        in_maps.append({"xk": xk, "wk": wk, "xsq": xsq_l, "v": v})
    return in_maps


def _gather(results):
    parts = []
    for c in range(NCORES):
        o = results[c]["out"]                              # [P, MT, O]
        parts.append(o.transpose(1, 0, 2).reshape(BL, O))
    return np.ascontiguousarray(np.concatenate(parts, axis=0))


def _run(in_maps, **kwargs):
    from concourse.bass_utils import run_bass_kernel_spmd

    nc = _get_nc()
    return run_bass_kernel_spmd(nc, in_maps, core_ids=list(range(NCORES)), **kwargs)


def kernel(x, weights, bias):
    in_maps = _prep_inputs(x, weights, bias)
    res = _run(in_maps)
    return _gather(res.results)


# revision 4
# speedup vs baseline: 1.1298x; 1.1274x over previous
"""Trainium2 Bass kernel for nn_MemLayer (retrieval_knn).

Math:  out[b,o] = -mean_d (x[b,d] - w[o,d])^2 + bias[o]
              =  (2/D) * (x @ w.T)[b,o]  -  ||x_b||^2/D  +  (bias[o] - ||w_o||^2/D)

Strategy:
  - Data-parallel shard x along batch across 8 NeuronCores (1024 rows each),
    replicate weights. No cross-core communication; gather outputs on host.
  - Per core: bf16 GEMM [1024,1024] @ [1024,4096] on the PE array with fp32
    PSUM accumulation. The 2/D scale is folded into the bf16 weights on host.
  - Schedule: n-tile outer; within an n-tile the contraction (kc) loop is
    OUTER across all 8 PSUM banks (one per m-tile), so the first matmuls only
    need one 256KB x-chunk + one 128KB w-chunk of DMA before the PE starts,
    and weight n-chunks stream in ~5x faster than the PE consumes them.
  - Corrections stay fp32, fused into PSUM eviction:
      * ACT:  out_sb = psum + xsq[p]   (per-partition bias, xsq = -||x||^2/D)
      * DVE:  out_sb += v[o]           (v = bias - ||w||^2/D, SBUF row bcast)
    then a 256KB DMA per (m,n) tile straight to DRAM (no output staging tail).

Only the tiny rank-1 reductions (x_sq, w_sq) are computed on the host in fp32;
the O(B*O*D) GEMM runs on the PE arrays in bf16 (elementwise output error
~1e-5 relative, fp32 accumulation).
"""

import numpy as np
import ml_dtypes

B, D, O = 8192, 1024, 4096
NCORES = 8
BL = B // NCORES     # 1024 rows per core
P = 128
KT = D // P          # 8 k-tiles
MT = BL // P         # 8 m-tiles
NTILE = 512          # one PSUM bank of fp32
NT = O // NTILE      # 8 n-tiles

_CACHE = {}


def _get_nc():
    if "nc" in _CACHE:
        return _CACHE["nc"]

    import concourse.bacc as bacc
    import concourse.tile as tile
    from concourse import mybir

    nc = bacc.Bacc("TRN2", target_bir_lowering=False)

    xk_d = nc.dram_tensor("xk", [P, KT, BL], mybir.dt.bfloat16, kind="ExternalInput")
    wk_d = nc.dram_tensor("wk", [NT, P, KT, NTILE], mybir.dt.bfloat16,
                          kind="ExternalInput")
    xsq_d = nc.dram_tensor("xsq", [P, MT], mybir.dt.float32, kind="ExternalInput")
    v_d = nc.dram_tensor("v", [1, O], mybir.dt.float32, kind="ExternalInput")
    out_d = nc.dram_tensor("out", [P, MT, O], mybir.dt.float32, kind="ExternalOutput")

    f32 = mybir.dt.float32
    bf16 = mybir.dt.bfloat16

    with tile.TileContext(nc) as tc:
        with (
            tc.tile_pool(name="const", bufs=1) as cpool,
            tc.tile_pool(name="psum", bufs=8, space="PSUM") as ppool,
            tc.tile_pool(name="outp", bufs=4) as opool,
        ):
            xk_sb = cpool.tile([P, KT, BL], bf16)
            wk_sb = cpool.tile([P, NT, KT, NTILE], bf16)
            xsq_sb = cpool.tile([P, MT], f32)
            vb_sb = cpool.tile([P, O], f32)

            # DMA issue order = queue priority (all sync-issued DMAs share the
            # 16 HWDGE FIFOs). The PE's first matmul needs only xk[:,0,:] +
            # wk[0,:,0,:]; interleave the nt=0 k-chunks first. Later weight
            # chunks are prefetched just-in-time from inside the nt loop so
            # they never sit ahead of output evictions in the FIFOs.
            for kc in range(KT):
                nc.sync.dma_start(out=xk_sb[:, kc, :], in_=xk_d[:, kc, :])
                nc.sync.dma_start(out=wk_sb[:, 0, kc, :], in_=wk_d[0, :, kc, :])
            nc.sync.dma_start(out=xsq_sb[:], in_=xsq_d[:])
            nc.sync.dma_start(out=vb_sb[:], in_=v_d[:].to_broadcast([P, O]))
            nc.sync.dma_start(out=wk_sb[:, 1, :, :], in_=wk_d[1])

            for nt in range(NT):
                if nt + 2 < NT:
                    nc.sync.dma_start(out=wk_sb[:, nt + 2, :, :], in_=wk_d[nt + 2])
                pss = []
                for mt in range(MT):
                    ps = ppool.tile([P, NTILE], f32, tag="ps")
                    pss.append(ps)
                for kc in range(KT):
                    for mt in range(MT):
                        nc.tensor.matmul(
                            pss[mt][:],
                            lhsT=xk_sb[:, kc, mt * P:(mt + 1) * P],
                            rhs=wk_sb[:, nt, kc, :],
                            start=(kc == 0),
                            stop=(kc == KT - 1),
                        )
                ns = slice(nt * NTILE, (nt + 1) * NTILE)
                for mt in range(MT):
                    ob = opool.tile([P, NTILE], f32)
                    nc.scalar.activation(
                        ob[:],
                        pss[mt][:],
                        mybir.ActivationFunctionType.Identity,
                        bias=xsq_sb[:, mt:mt + 1],
                    )
                    nc.vector.tensor_add(ob[:], ob[:], vb_sb[:, ns])
                    nc.sync.dma_start(out=out_d[:, mt, ns], in_=ob[:])

    nc.finalize()
    _CACHE["nc"] = nc
    return nc


def _prep_inputs(x, weights, bias):
    """Shard + lay out host inputs -> per-core in_maps."""
    bf16 = ml_dtypes.bfloat16
    x = np.asarray(x, dtype=np.float32)
    weights = np.asarray(weights, dtype=np.float32)
    bias = np.asarray(bias, dtype=np.float32)

    # [D, O] with 2/D folded in -> [NT, P, KT, NTILE]
    wT = weights.T * np.float32(2.0 / D)
    wk = np.ascontiguousarray(
        wT.reshape(KT, P, NT, NTILE).transpose(2, 1, 0, 3).astype(bf16)
    )
    w_sq = np.einsum("od,od->o", weights, weights)
    v = np.ascontiguousarray((bias - w_sq / np.float32(D)).reshape(1, O))

    in_maps = []
    for c in range(NCORES):
        xs = x[c * BL:(c + 1) * BL]                       # [BL, D] fp32
        xT = xs.T                                          # [D, BL]
        xk = np.ascontiguousarray(
            xT.reshape(KT, P, BL).transpose(1, 0, 2).astype(bf16)
        )
        xsq = -np.einsum("bd,bd->b", xs, xs) / np.float32(D)   # [BL]
        xsq_l = np.ascontiguousarray(xsq.reshape(MT, P).T)     # [P, MT]
        in_maps.append({"xk": xk, "wk": wk, "xsq": xsq_l, "v": v})
    return in_maps


def _gather(results):
    parts = []
    for c in range(NCORES):
        o = results[c]["out"]                              # [P, MT, O]
        parts.append(o.transpose(1, 0, 2).reshape(BL, O))
    return np.ascontiguousarray(np.concatenate(parts, axis=0))


def _run(in_maps, **kwargs):
    from concourse.bass_utils import run_bass_kernel_spmd

    nc = _get_nc()
    return run_bass_kernel_spmd(nc, in_maps, core_ids=list(range(NCORES)), **kwargs)


def kernel(x, weights, bias):
    in_maps = _prep_inputs(x, weights, bias)
    res = _run(in_maps)
    return _gather(res.results)


# revision 5
# speedup vs baseline: 1.6951x; 1.5004x over previous
"""Trainium2 Bass kernel for nn_MemLayer (retrieval_knn).

Math:  out[b,o] = -mean_d (x[b,d] - w[o,d])^2 + bias[o]
              =  s * (x' @ w'.T)[b,o]  -  ||x_b||^2/D  +  (bias[o] - ||w_o||^2/D)

  with x' = 16*x, w' = 4096*w in fp8e4m3 and s = 2/(D*16*4096) applied on the
  ACT engine at PSUM eviction (both scale factors keep the fp8 operands inside
  the e4m3 normal range; accumulation is fp32 in PSUM).

Strategy:
  - Data-parallel shard x along batch across 8 NeuronCores (1024 rows each),
    replicate weights. No cross-core communication; gather outputs on host.
  - Per core: fp8 GEMM [1024,1024] @ [1024,4096] using DoubleRow perf mode
    (2 fp8 weights per PE cell -> contraction 256 per matmul, 256 matmuls).
  - Schedule: n-tile outer; within an n-tile the contraction (kd) loop is
    OUTER across all 8 PSUM banks (one per m-tile), so the first matmuls only
    need a few hundred KB of DMA before the PE starts. Weight n-chunks are
    prefetched just-in-time from inside the nt loop so they never sit ahead
    of output evictions in the shared HWDGE FIFOs.
  - Corrections stay fp32, fused into PSUM eviction:
      * ACT:  out_sb = psum * s + xsq[p]   (per-partition bias, -||x||^2/D)
      * DVE:  out_sb += v[o]               (v = bias - ||w||^2/D, row bcast)
    then a 256KB DMA per (m,n) tile straight to DRAM.

The rank-1 reductions (x_sq, w_sq) are computed on the host in fp32, so the
only reduced-precision term is the (2/D)*x.w correction, which is ~1e-3 of
the output scale; elementwise output error stays ~3e-5 relative.
"""

import numpy as np
import ml_dtypes

B, D, O = 8192, 1024, 4096
NCORES = 8
BL = B // NCORES     # 1024 rows per core
P = 128
MT = BL // P         # 8 m-tiles
NTILE = 512          # one PSUM bank of fp32
NT = O // NTILE      # 8 n-tiles

FP8 = True
KT = D // P          # 8 k-tiles (bf16 path)
KD = D // (2 * P)    # 4 double-k-tiles (fp8 DoubleRow path)
XSCALE = 16.0        # x -> fp8 pre-scale
WSCALE = 4096.0      # w -> fp8 pre-scale

_CACHE = {}


def _get_nc():
    key = ("nc", FP8)
    if key in _CACHE:
        return _CACHE[key]

    import concourse.bacc as bacc
    import concourse.tile as tile
    from concourse import mybir

    nc = bacc.Bacc("TRN2", target_bir_lowering=False)

    f32 = mybir.dt.float32
    mm_dt = mybir.dt.float8e4 if FP8 else mybir.dt.bfloat16

    if FP8:
        xk_d = nc.dram_tensor("xk", [P, KD, 2, BL], mm_dt, kind="ExternalInput")
        wk_d = nc.dram_tensor("wk", [NT, P, KD, 2, NTILE], mm_dt,
                              kind="ExternalInput")
    else:
        xk_d = nc.dram_tensor("xk", [P, KT, BL], mm_dt, kind="ExternalInput")
        wk_d = nc.dram_tensor("wk", [NT, P, KT, NTILE], mm_dt,
                              kind="ExternalInput")
    xsq_d = nc.dram_tensor("xsq", [P, MT], f32, kind="ExternalInput")
    v_d = nc.dram_tensor("v", [1, O], f32, kind="ExternalInput")
    out_d = nc.dram_tensor("out", [P, MT, O], f32, kind="ExternalOutput")

    act_scale = float(2.0 / (D * XSCALE * WSCALE)) if FP8 else 1.0
    kiters = KD if FP8 else KT

    with tile.TileContext(nc) as tc:
        with (
            tc.tile_pool(name="const", bufs=1) as cpool,
            tc.tile_pool(name="psum", bufs=8, space="PSUM") as ppool,
            tc.tile_pool(name="outp", bufs=6) as opool,
        ):
            if FP8:
                xk_sb = cpool.tile([P, KD, 2, BL], mm_dt)
                wk_sb = cpool.tile([P, NT, KD, 2, NTILE], mm_dt)
            else:
                xk_sb = cpool.tile([P, KT, BL], mm_dt)
                wk_sb = cpool.tile([P, NT, KT, NTILE], mm_dt)
            xsq_sb = cpool.tile([P, MT], f32)
            vb_sb = cpool.tile([P, O], f32)

            for kc in range(kiters):
                if FP8:
                    nc.sync.dma_start(out=xk_sb[:, kc, :, :], in_=xk_d[:, kc])
                    nc.sync.dma_start(out=wk_sb[:, 0, kc, :, :],
                                      in_=wk_d[0, :, kc])
                else:
                    nc.sync.dma_start(out=xk_sb[:, kc, :], in_=xk_d[:, kc, :])
                    nc.sync.dma_start(out=wk_sb[:, 0, kc, :],
                                      in_=wk_d[0, :, kc, :])
            nc.sync.dma_start(out=xsq_sb[:], in_=xsq_d[:])
            nc.sync.dma_start(out=vb_sb[:], in_=v_d[:].to_broadcast([P, O]))
            nc.sync.dma_start(out=wk_sb[:, 1], in_=wk_d[1])

            for nt in range(NT):
                if nt + 2 < NT:
                    nc.sync.dma_start(out=wk_sb[:, nt + 2], in_=wk_d[nt + 2])
                pss = []
                for mt in range(MT):
                    ps = ppool.tile([P, NTILE], f32, tag="ps")
                    pss.append(ps)
                for kc in range(kiters):
                    for mt in range(MT):
                        if FP8:
                            nc.tensor.matmul(
                                pss[mt][:],
                                lhsT=xk_sb[:, kc, :, mt * P:(mt + 1) * P],
                                rhs=wk_sb[:, nt, kc, :, :],
                                start=(kc == 0),
                                stop=(kc == kiters - 1),
                                perf_mode=mybir.MatmulPerfMode.DoubleRow,
                            )
                        else:
                            nc.tensor.matmul(
                                pss[mt][:],
                                lhsT=xk_sb[:, kc, mt * P:(mt + 1) * P],
                                rhs=wk_sb[:, nt, kc, :],
                                start=(kc == 0),
                                stop=(kc == kiters - 1),
                            )
                ns = slice(nt * NTILE, (nt + 1) * NTILE)
                for mt in range(MT):
                    ob = opool.tile([P, NTILE], f32)
                    nc.scalar.activation(
                        ob[:],
                        pss[mt][:],
                        mybir.ActivationFunctionType.Identity,
                        bias=xsq_sb[:, mt:mt + 1],
                        scale=act_scale,
                    )
                    nc.vector.tensor_add(ob[:], ob[:], vb_sb[:, ns])
                    nc.sync.dma_start(out=out_d[:, mt, ns], in_=ob[:])

    nc.finalize()
    _CACHE[key] = nc
    return nc


def _prep_inputs(x, weights, bias):
    """Shard + lay out host inputs -> per-core in_maps."""
    x = np.asarray(x, dtype=np.float32)
    weights = np.asarray(weights, dtype=np.float32)
    bias = np.asarray(bias, dtype=np.float32)

    w_sq = np.einsum("od,od->o", weights, weights)
    v = np.ascontiguousarray((bias - w_sq / np.float32(D)).reshape(1, O))

    if FP8:
        dt = ml_dtypes.float8_e4m3
        # k = kd*256 + i*128 + p
        wT = weights.T * np.float32(WSCALE)                   # [D, O]
        wk = np.ascontiguousarray(
            wT.reshape(KD, 2, P, NT, NTILE)
            .transpose(3, 2, 0, 1, 4)
            .astype(dt)
        )
    else:
        dt = ml_dtypes.bfloat16
        wT = weights.T * np.float32(2.0 / D)
        wk = np.ascontiguousarray(
            wT.reshape(KT, P, NT, NTILE).transpose(2, 1, 0, 3).astype(dt)
        )

    in_maps = []
    for c in range(NCORES):
        xs = x[c * BL:(c + 1) * BL]                            # [BL, D] fp32
        xT = xs.T                                              # [D, BL]
        if FP8:
            xk = np.ascontiguousarray(
                (xT.reshape(KD, 2, P, BL) * np.float32(XSCALE))
                .transpose(2, 0, 1, 3)
                .astype(dt)
            )
        else:
            xk = np.ascontiguousarray(
                xT.reshape(KT, P, BL).transpose(1, 0, 2).astype(dt)
            )
        xsq = -np.einsum("bd,bd->b", xs, xs) / np.float32(D)   # [BL]
        xsq_l = np.ascontiguousarray(xsq.reshape(MT, P).T)     # [P, MT]
        in_maps.append({"xk": xk, "wk": wk, "xsq": xsq_l, "v": v})
    return in_maps


def _gather(results):
    parts = []
    for c in range(NCORES):
        o = results[c]["out"]                                  # [P, MT, O]
        parts.append(o.transpose(1, 0, 2).reshape(BL, O))
    return np.ascontiguousarray(np.concatenate(parts, axis=0))


def _run(in_maps, **kwargs):
    from concourse.bass_utils import run_bass_kernel_spmd

    nc = _get_nc()
    return run_bass_kernel_spmd(nc, in_maps, core_ids=list(range(NCORES)), **kwargs)


def kernel(x, weights, bias):
    in_maps = _prep_inputs(x, weights, bias)
    res = _run(in_maps)
    return _gather(res.results)


# revision 7
# speedup vs baseline: 1.7341x; 1.0230x over previous
"""Trainium2 Bass kernel for nn_MemLayer (retrieval_knn).

Math:  out[b,o] = -mean_d (x[b,d] - w[o,d])^2 + bias[o]
              =  s * (x' @ w'.T)[b,o]  -  ||x_b||^2/D  +  (bias[o] - ||w_o||^2/D)

  with x' = 16*x, w' = 4096*w in fp8e4m3 and s = 2/(D*16*4096) applied on the
  ACT engine at PSUM eviction (both scale factors keep the fp8 operands inside
  the e4m3 normal range; accumulation is fp32 in PSUM).

Strategy:
  - Data-parallel shard x along batch across 8 NeuronCores (1024 rows each),
    replicate weights. No cross-core communication; gather outputs on host.
  - Per core: fp8 GEMM [1024,1024] @ [1024,4096] using DoubleRow perf mode
    (2 fp8 weights per PE cell -> contraction 256 per matmul, 256 matmuls).
  - Schedule: n-tile outer; within an n-tile the contraction (kd) loop is
    OUTER across all 8 PSUM banks (one per m-tile), so the first matmuls only
    need a few hundred KB of DMA before the PE starts. Weight n-chunks are
    prefetched just-in-time from inside the nt loop so they never sit ahead
    of output evictions in the shared HWDGE FIFOs.
  - Corrections stay fp32, fused into PSUM eviction:
      * ACT:  out_sb = psum * s + xsq[p]   (per-partition bias, -||x||^2/D)
      * DVE:  out_sb += v[o]               (v = bias - ||w||^2/D, row bcast)
    then a 256KB DMA per (m,n) tile straight to DRAM.

The rank-1 reductions (x_sq, w_sq) are computed on the host in fp32, so the
only reduced-precision term is the (2/D)*x.w correction, which is ~1e-3 of
the output scale; elementwise output error stays ~3e-5 relative.
"""

import numpy as np
import ml_dtypes

B, D, O = 8192, 1024, 4096
NCORES = 8
BL = B // NCORES     # 1024 rows per core
P = 128
MT = BL // P         # 8 m-tiles
NTILE = 512          # one PSUM bank of fp32
NT = O // NTILE      # 8 n-tiles

FP8 = True
KT = D // P          # 8 k-tiles (bf16 path)
KD = D // (2 * P)    # 4 double-k-tiles (fp8 DoubleRow path)
XSCALE = 16.0        # x -> fp8 pre-scale
WSCALE = 4096.0      # w -> fp8 pre-scale

_CACHE = {}


def _get_nc():
    key = ("nc", FP8)
    if key in _CACHE:
        return _CACHE[key]

    import concourse.bacc as bacc
    import concourse.tile as tile
    from concourse import mybir

    nc = bacc.Bacc("TRN2", target_bir_lowering=False)

    f32 = mybir.dt.float32
    mm_dt = mybir.dt.float8e4 if FP8 else mybir.dt.bfloat16

    if FP8:
        xk_d = nc.dram_tensor("xk", [P, KD, 2, BL], mm_dt, kind="ExternalInput")
        wk_d = nc.dram_tensor("wk", [NT, P, KD, 2, NTILE], mm_dt,
                              kind="ExternalInput")
    else:
        xk_d = nc.dram_tensor("xk", [P, KT, BL], mm_dt, kind="ExternalInput")
        wk_d = nc.dram_tensor("wk", [NT, P, KT, NTILE], mm_dt,
                              kind="ExternalInput")
    xsq_d = nc.dram_tensor("xsq", [P, MT], f32, kind="ExternalInput")
    v_d = nc.dram_tensor("v", [1, O], f32, kind="ExternalInput")
    out_d = nc.dram_tensor("out", [P, MT, O], f32, kind="ExternalOutput")

    act_scale = float(2.0 / (D * XSCALE * WSCALE)) if FP8 else 1.0
    kiters = KD if FP8 else KT

    with tile.TileContext(nc) as tc:
        with (
            tc.tile_pool(name="const", bufs=1) as cpool,
            tc.tile_pool(name="psum", bufs=8, space="PSUM") as ppool,
            tc.tile_pool(name="outp", bufs=6) as opool,
        ):
            if FP8:
                xk_sb = cpool.tile([P, KD, 2, BL], mm_dt)
                wk_sb = cpool.tile([P, NT, KD, 2, NTILE], mm_dt)
            else:
                xk_sb = cpool.tile([P, KT, BL], mm_dt)
                wk_sb = cpool.tile([P, NT, KT, NTILE], mm_dt)
            xsq_sb = cpool.tile([P, MT], f32)
            vb_sb = cpool.tile([P, O], f32)

            # Warm-up: the PE HAM clock gate needs ~3.4us of sustained matmul
            # activity to unthrottle 1.2 -> 2.4 GHz. The PE is otherwise idle
            # while the first input chunks DMA in, so burn that window with
            # short matmuls on a zeroed tile; the real matmuls then start at
            # full clock. Keep the total under the DMA head so they never
            # delay real work (PE executes its queue in program order).
            zk = cpool.tile([P, 2, 64], mm_dt)
            nc.vector.memset(zk[:], 0.0)
            ps_warm = ppool.tile([P, NTILE], f32, tag="ps")
            for _ in range(48):
                if FP8:
                    nc.tensor.matmul(
                        ps_warm[:64, :64],
                        lhsT=zk[:],
                        rhs=zk[:],
                        start=True,
                        stop=True,
                        perf_mode=mybir.MatmulPerfMode.DoubleRow,
                    )
                else:
                    nc.tensor.matmul(
                        ps_warm[:64, :64],
                        lhsT=zk[:, 0, :],
                        rhs=zk[:, 0, :],
                        start=True,
                        stop=True,
                    )

            for kc in range(kiters):
                if FP8:
                    nc.sync.dma_start(out=xk_sb[:, kc, :, :], in_=xk_d[:, kc])
                    nc.sync.dma_start(out=wk_sb[:, 0, kc, :, :],
                                      in_=wk_d[0, :, kc])
                else:
                    nc.sync.dma_start(out=xk_sb[:, kc, :], in_=xk_d[:, kc, :])
                    nc.sync.dma_start(out=wk_sb[:, 0, kc, :],
                                      in_=wk_d[0, :, kc, :])
            nc.sync.dma_start(out=xsq_sb[:], in_=xsq_d[:])
            nc.sync.dma_start(out=wk_sb[:, 1], in_=wk_d[1])
            nc.sync.dma_start(out=vb_sb[:], in_=v_d[:].to_broadcast([P, O]))
            nc.sync.dma_start(out=wk_sb[:, 2], in_=wk_d[2])

            for nt in range(NT):
                if nt + 3 < NT:
                    nc.sync.dma_start(out=wk_sb[:, nt + 3], in_=wk_d[nt + 3])
                pss = []
                for mt in range(MT):
                    ps = ppool.tile([P, NTILE], f32, tag="ps")
                    pss.append(ps)
                for kc in range(kiters):
                    for mt in range(MT):
                        if FP8:
                            nc.tensor.matmul(
                                pss[mt][:],
                                lhsT=xk_sb[:, kc, :, mt * P:(mt + 1) * P],
                                rhs=wk_sb[:, nt, kc, :, :],
                                start=(kc == 0),
                                stop=(kc == kiters - 1),
                                perf_mode=mybir.MatmulPerfMode.DoubleRow,
                            )
                        else:
                            nc.tensor.matmul(
                                pss[mt][:],
                                lhsT=xk_sb[:, kc, mt * P:(mt + 1) * P],
                                rhs=wk_sb[:, nt, kc, :],
                                start=(kc == 0),
                                stop=(kc == kiters - 1),
                            )
                ns = slice(nt * NTILE, (nt + 1) * NTILE)
                for mt in range(MT):
                    ob = opool.tile([P, NTILE], f32)
                    nc.scalar.activation(
                        ob[:],
                        pss[mt][:],
                        mybir.ActivationFunctionType.Identity,
                        bias=xsq_sb[:, mt:mt + 1],
                        scale=act_scale,
                    )
                    nc.vector.tensor_add(ob[:], ob[:], vb_sb[:, ns])
                    nc.sync.dma_start(out=out_d[:, mt, ns], in_=ob[:])

    nc.finalize()
    _CACHE[key] = nc
    return nc


def _prep_inputs(x, weights, bias):
    """Shard + lay out host inputs -> per-core in_maps."""
    x = np.asarray(x, dtype=np.float32)
    weights = np.asarray(weights, dtype=np.float32)
    bias = np.asarray(bias, dtype=np.float32)

    w_sq = np.einsum("od,od->o", weights, weights)
    v = np.ascontiguousarray((bias - w_sq / np.float32(D)).reshape(1, O))

    if FP8:
        dt = ml_dtypes.float8_e4m3
        # k = kd*256 + i*128 + p
        wT = weights.T * np.float32(WSCALE)                   # [D, O]
        wk = np.ascontiguousarray(
            wT.reshape(KD, 2, P, NT, NTILE)
            .transpose(3, 2, 0, 1, 4)
            .astype(dt)
        )
    else:
        dt = ml_dtypes.bfloat16
        wT = weights.T * np.float32(2.0 / D)
        wk = np.ascontiguousarray(
            wT.reshape(KT, P, NT, NTILE).transpose(2, 1, 0, 3).astype(dt)
        )

    in_maps = []
    for c in range(NCORES):
        xs = x[c * BL:(c + 1) * BL]                            # [BL, D] fp32
        xT = xs.T                                              # [D, BL]
        if FP8:
            xk = np.ascontiguousarray(
                (xT.reshape(KD, 2, P, BL) * np.float32(XSCALE))
                .transpose(2, 0, 1, 3)
                .astype(dt)
            )
        else:
            xk = np.ascontiguousarray(
                xT.reshape(KT, P, BL).transpose(1, 0, 2).astype(dt)
            )
        xsq = -np.einsum("bd,bd->b", xs, xs) / np.float32(D)   # [BL]
        xsq_l = np.ascontiguousarray(xsq.reshape(MT, P).T)     # [P, MT]
        in_maps.append({"xk": xk, "wk": wk, "xsq": xsq_l, "v": v})
    return in_maps


def _gather(results):
    parts = []
    for c in range(NCORES):
        o = results[c]["out"]                                  # [P, MT, O]
        parts.append(o.transpose(1, 0, 2).reshape(BL, O))
    return np.ascontiguousarray(np.concatenate(parts, axis=0))


def _run(in_maps, **kwargs):
    from concourse.bass_utils import run_bass_kernel_spmd

    nc = _get_nc()
    return run_bass_kernel_spmd(nc, in_maps, core_ids=list(range(NCORES)), **kwargs)


def kernel(x, weights, bias):
    in_maps = _prep_inputs(x, weights, bias)
    res = _run(in_maps)
    return _gather(res.results)


# revision 9
# speedup vs baseline: 1.7453x; 1.0064x over previous
"""Trainium2 Bass kernel for nn_MemLayer (retrieval_knn).

Math:  out[b,o] = -mean_d (x[b,d] - w[o,d])^2 + bias[o]
              =  s * (x' @ w'.T)[b,o]  -  ||x_b||^2/D  +  (bias[o] - ||w_o||^2/D)

  with x' = 16*x, w' = 4096*w in fp8e4m3 and s = 2/(D*16*4096) applied on the
  ACT engine at PSUM eviction (both scale factors keep the fp8 operands inside
  the e4m3 normal range; accumulation is fp32 in PSUM).

Strategy:
  - Data-parallel shard x along batch across 8 NeuronCores (1024 rows each),
    replicate weights. No cross-core communication; gather outputs on host.
  - Per core: fp8 GEMM [1024,1024] @ [1024,4096] using DoubleRow perf mode
    (2 fp8 weights per PE cell -> contraction 256 per matmul, 256 matmuls).
  - Schedule: n-tile outer; within an n-tile the contraction (kd) loop is
    OUTER across all 8 PSUM banks (one per m-tile), so the first matmuls only
    need a few hundred KB of DMA before the PE starts. Weight n-chunks are
    prefetched just-in-time from inside the nt loop so they never sit ahead
    of output evictions in the shared HWDGE FIFOs.
  - Corrections stay fp32, fused into PSUM eviction:
      * ACT:  out_sb = psum * s + xsq[p]   (per-partition bias, -||x||^2/D)
      * DVE:  out_sb += v[o]               (v = bias - ||w||^2/D, row bcast)
    then a 256KB DMA per (m,n) tile straight to DRAM.

The rank-1 reductions (x_sq, w_sq) are computed on the host in fp32, so the
only reduced-precision term is the (2/D)*x.w correction, which is ~1e-3 of
the output scale; elementwise output error stays ~3e-5 relative.
"""

import numpy as np
import ml_dtypes

B, D, O = 8192, 1024, 4096
NCORES = 8
BL = B // NCORES     # 1024 rows per core
P = 128
MT = BL // P         # 8 m-tiles
NTILE = 512          # one PSUM bank of fp32
NT = O // NTILE      # 8 n-tiles

FP8 = True
KT = D // P          # 8 k-tiles (bf16 path)
KD = D // (2 * P)    # 4 double-k-tiles (fp8 DoubleRow path)
XSCALE = 16.0        # x -> fp8 pre-scale
WSCALE = 4096.0      # w -> fp8 pre-scale

_CACHE = {}


def _get_nc():
    key = ("nc", FP8)
    if key in _CACHE:
        return _CACHE[key]

    import concourse.bacc as bacc
    import concourse.tile as tile
    from concourse import mybir

    nc = bacc.Bacc("TRN2", target_bir_lowering=False)

    f32 = mybir.dt.float32
    mm_dt = mybir.dt.float8e4 if FP8 else mybir.dt.bfloat16

    if FP8:
        xk_d = nc.dram_tensor("xk", [P, KD, 2, BL], mm_dt, kind="ExternalInput")
        wk_d = nc.dram_tensor("wk", [NT, P, KD, 2, NTILE], mm_dt,
                              kind="ExternalInput")
    else:
        xk_d = nc.dram_tensor("xk", [P, KT, BL], mm_dt, kind="ExternalInput")
        wk_d = nc.dram_tensor("wk", [NT, P, KT, NTILE], mm_dt,
                              kind="ExternalInput")
    xsq_d = nc.dram_tensor("xsq", [P, MT], f32, kind="ExternalInput")
    v_d = nc.dram_tensor("v", [1, O], f32, kind="ExternalInput")
    out_d = nc.dram_tensor("out", [P, MT, O], f32, kind="ExternalOutput")

    act_scale = float(2.0 / (D * XSCALE * WSCALE)) if FP8 else 1.0
    kiters = KD if FP8 else KT

    with tile.TileContext(nc) as tc:
        with (
            tc.tile_pool(name="const", bufs=1) as cpool,
            tc.tile_pool(name="psum", bufs=8, space="PSUM") as ppool,
            tc.tile_pool(name="outp", bufs=6) as opool,
        ):
            if FP8:
                xk_sb = cpool.tile([P, KD, 2, BL], mm_dt)
                wk_sb = cpool.tile([P, NT, KD, 2, NTILE], mm_dt)
            else:
                xk_sb = cpool.tile([P, KT, BL], mm_dt)
                wk_sb = cpool.tile([P, NT, KT, NTILE], mm_dt)
            xsq_sb = cpool.tile([P, MT], f32)
            vb_sb = cpool.tile([P, O], f32)

            # Warm-up: the PE HAM clock gate needs ~3.4us of sustained matmul
            # activity to unthrottle 1.2 -> 2.4 GHz. The PE is otherwise idle
            # while the first input chunks DMA in, so burn that window with
            # short matmuls on a zeroed tile; the real matmuls then start at
            # full clock. Keep the total under the DMA head so they never
            # delay real work (PE executes its queue in program order).
            zk = cpool.tile([P, 2, 64], mm_dt)
            nc.gpsimd.memset(zk[:], 0.0)
            ps_warm = ppool.tile([P, NTILE], f32, tag="ps")
            for _ in range(30):
                if FP8:
                    nc.tensor.matmul(
                        ps_warm[:64, :64],
                        lhsT=zk[:],
                        rhs=zk[:],
                        start=True,
                        stop=True,
                        perf_mode=mybir.MatmulPerfMode.DoubleRow,
                    )
                else:
                    nc.tensor.matmul(
                        ps_warm[:64, :64],
                        lhsT=zk[:, 0, :],
                        rhs=zk[:, 0, :],
                        start=True,
                        stop=True,
                    )

            for kc in range(kiters):
                if FP8:
                    nc.sync.dma_start(out=xk_sb[:, kc, :, :], in_=xk_d[:, kc])
                    nc.sync.dma_start(out=wk_sb[:, 0, kc, :, :],
                                      in_=wk_d[0, :, kc])
                else:
                    nc.sync.dma_start(out=xk_sb[:, kc, :], in_=xk_d[:, kc, :])
                    nc.sync.dma_start(out=wk_sb[:, 0, kc, :],
                                      in_=wk_d[0, :, kc, :])
            nc.sync.dma_start(out=xsq_sb[:], in_=xsq_d[:])
            nc.sync.dma_start(out=wk_sb[:, 1], in_=wk_d[1])
            nc.sync.dma_start(out=vb_sb[:], in_=v_d[:].to_broadcast([P, O]))
            nc.sync.dma_start(out=wk_sb[:, 2], in_=wk_d[2])

            # Per n-tile, process the 8 m-tiles as two half-passes of 4 PSUM
            # banks: the PE accumulates into one half while the ACT/DVE
            # eviction chain drains the other (serial ACT frees banks at
            # ~0.7us/bank, slower than the PE's first-kc-pass consumption, so
            # a full 8-bank rotation stalls the PE at every nt boundary).
            for nt in range(NT):
                if nt + 3 < NT:
                    nc.sync.dma_start(out=wk_sb[:, nt + 3], in_=wk_d[nt + 3])
                ns = slice(nt * NTILE, (nt + 1) * NTILE)
                for half in range(2):
                    mts = range(half * (MT // 2), (half + 1) * (MT // 2))
                    pss = {}
                    for mt in mts:
                        ps = ppool.tile([P, NTILE], f32, tag="ps")
                        pss[mt] = ps
                    for kc in range(kiters):
                        for mt in mts:
                            if FP8:
                                nc.tensor.matmul(
                                    pss[mt][:],
                                    lhsT=xk_sb[:, kc, :, mt * P:(mt + 1) * P],
                                    rhs=wk_sb[:, nt, kc, :, :],
                                    start=(kc == 0),
                                    stop=(kc == kiters - 1),
                                    perf_mode=mybir.MatmulPerfMode.DoubleRow,
                                )
                            else:
                                nc.tensor.matmul(
                                    pss[mt][:],
                                    lhsT=xk_sb[:, kc, mt * P:(mt + 1) * P],
                                    rhs=wk_sb[:, nt, kc, :],
                                    start=(kc == 0),
                                    stop=(kc == kiters - 1),
                                )
                    for mt in mts:
                        ob = opool.tile([P, NTILE], f32)
                        nc.scalar.activation(
                            ob[:],
                            pss[mt][:],
                            mybir.ActivationFunctionType.Identity,
                            bias=xsq_sb[:, mt:mt + 1],
                            scale=act_scale,
                        )
                        nc.vector.tensor_add(ob[:], ob[:], vb_sb[:, ns])
                        nc.sync.dma_start(out=out_d[:, mt, ns], in_=ob[:])

    nc.finalize()
    _CACHE[key] = nc
    return nc


def _prep_inputs(x, weights, bias):
    """Shard + lay out host inputs -> per-core in_maps."""
    x = np.asarray(x, dtype=np.float32)
    weights = np.asarray(weights, dtype=np.float32)
    bias = np.asarray(bias, dtype=np.float32)

    w_sq = np.einsum("od,od->o", weights, weights)
    v = np.ascontiguousarray((bias - w_sq / np.float32(D)).reshape(1, O))

    if FP8:
        dt = ml_dtypes.float8_e4m3
        # k = kd*256 + i*128 + p
        wT = weights.T * np.float32(WSCALE)                   # [D, O]
        wk = np.ascontiguousarray(
            wT.reshape(KD, 2, P, NT, NTILE)
            .transpose(3, 2, 0, 1, 4)
            .astype(dt)
        )
    else:
        dt = ml_dtypes.bfloat16
        wT = weights.T * np.float32(2.0 / D)
        wk = np.ascontiguousarray(
            wT.reshape(KT, P, NT, NTILE).transpose(2, 1, 0, 3).astype(dt)
        )

    in_maps = []
    for c in range(NCORES):
        xs = x[c * BL:(c + 1) * BL]                            # [BL, D] fp32
        xT = xs.T                                              # [D, BL]
        if FP8:
            xk = np.ascontiguousarray(
                (xT.reshape(KD, 2, P, BL) * np.float32(XSCALE))
                .transpose(2, 0, 1, 3)
                .astype(dt)
            )
        else:
            xk = np.ascontiguousarray(
                xT.reshape(KT, P, BL).transpose(1, 0, 2).astype(dt)
            )
        xsq = -np.einsum("bd,bd->b", xs, xs) / np.float32(D)   # [BL]
        xsq_l = np.ascontiguousarray(xsq.reshape(MT, P).T)     # [P, MT]
        in_maps.append({"xk": xk, "wk": wk, "xsq": xsq_l, "v": v})
    return in_maps


def _gather(results):
    parts = []
    for c in range(NCORES):
        o = results[c]["out"]                                  # [P, MT, O]
        parts.append(o.transpose(1, 0, 2).reshape(BL, O))
    return np.ascontiguousarray(np.concatenate(parts, axis=0))


def _run(in_maps, **kwargs):
    from concourse.bass_utils import run_bass_kernel_spmd

    nc = _get_nc()
    return run_bass_kernel_spmd(nc, in_maps, core_ids=list(range(NCORES)), **kwargs)


def kernel(x, weights, bias):
    in_maps = _prep_inputs(x, weights, bias)
    res = _run(in_maps)
    return _gather(res.results)


# revision 10
# speedup vs baseline: 1.8467x; 1.0581x over previous
"""Trainium2 Bass kernel for nn_MemLayer (retrieval_knn).

Math:  out[b,o] = -mean_d (x[b,d] - w[o,d])^2 + bias[o]
              =  s * (x' @ w'.T)[b,o]  -  ||x_b||^2/D  +  (bias[o] - ||w_o||^2/D)

  with x' = 16*x, w' = 4096*w in fp8e4m3 and s = 2/(D*16*4096) applied on the
  ACT engine at PSUM eviction (both scale factors keep the fp8 operands inside
  the e4m3 normal range; accumulation is fp32 in PSUM).

Strategy:
  - Data-parallel shard x along batch across 8 NeuronCores (1024 rows each),
    replicate weights. No cross-core communication; gather outputs on host.
  - Per core: fp8 GEMM [1024,1024] @ [1024,4096] using DoubleRow perf mode
    (2 fp8 weights per PE cell -> contraction 256 per matmul, 256 matmuls).
  - Schedule: n-tile outer; within an n-tile the contraction (kd) loop is
    OUTER across all 8 PSUM banks (one per m-tile), so the first matmuls only
    need a few hundred KB of DMA before the PE starts. Weight n-chunks are
    prefetched just-in-time from inside the nt loop so they never sit ahead
    of output evictions in the shared HWDGE FIFOs.
  - Corrections stay fp32, fused into PSUM eviction:
      * ACT:  out_sb = psum * s + xsq[p]   (per-partition bias, -||x||^2/D)
      * DVE:  out_sb += v[o]               (v = bias - ||w||^2/D, row bcast)
    then a 256KB DMA per (m,n) tile straight to DRAM.

The rank-1 reductions (x_sq, w_sq) are computed on the host in fp32, so the
only reduced-precision term is the (2/D)*x.w correction, which is ~1e-3 of
the output scale; elementwise output error stays ~3e-5 relative.
"""

import numpy as np
import ml_dtypes

B, D, O = 8192, 1024, 4096
NCORES = 8
BL = B // NCORES     # 1024 rows per core
P = 128
MT = BL // P         # 8 m-tiles
NTILE = 512          # one PSUM bank of fp32
NT = O // NTILE      # 8 n-tiles

FP8 = True
KT = D // P          # 8 k-tiles (bf16 path)
KD = D // (2 * P)    # 4 double-k-tiles (fp8 DoubleRow path)
XSCALE = 16.0        # x -> fp8 pre-scale
WSCALE = 4096.0      # w -> fp8 pre-scale

_CACHE = {}


def _get_nc():
    key = ("nc", FP8)
    if key in _CACHE:
        return _CACHE[key]

    import concourse.bacc as bacc
    import concourse.tile as tile
    from concourse import mybir

    nc = bacc.Bacc("TRN2", target_bir_lowering=False)

    f32 = mybir.dt.float32
    mm_dt = mybir.dt.float8e4 if FP8 else mybir.dt.bfloat16

    if FP8:
        xk_d = nc.dram_tensor("xk", [P, KD, 2, BL], mm_dt, kind="ExternalInput")
        wk_d = nc.dram_tensor("wk", [NT, P, KD, 2, NTILE], mm_dt,
                              kind="ExternalInput")
    else:
        xk_d = nc.dram_tensor("xk", [P, KT, BL], mm_dt, kind="ExternalInput")
        wk_d = nc.dram_tensor("wk", [NT, P, KT, NTILE], mm_dt,
                              kind="ExternalInput")
    xsq_d = nc.dram_tensor("xsq", [P, MT], f32, kind="ExternalInput")
    v_d = nc.dram_tensor("v", [1, O], f32, kind="ExternalInput")
    out_d = nc.dram_tensor("out", [P, MT, O], f32, kind="ExternalOutput")

    act_scale = float(2.0 / (D * XSCALE * WSCALE)) if FP8 else 1.0
    kiters = KD if FP8 else KT

    with tile.TileContext(nc) as tc:
        with (
            tc.tile_pool(name="const", bufs=1) as cpool,
            tc.tile_pool(name="psum", bufs=8, space="PSUM") as ppool,
            tc.tile_pool(name="outp", bufs=4) as opool,
        ):
            if FP8:
                xk_sb = cpool.tile([P, KD, 2, BL], mm_dt)
                wk_sb = cpool.tile([P, NT, KD, 2, NTILE], mm_dt)
            else:
                xk_sb = cpool.tile([P, KT, BL], mm_dt)
                wk_sb = cpool.tile([P, NT, KT, NTILE], mm_dt)
            xsq_sb = cpool.tile([P, MT], f32)
            vb_sb = cpool.tile([P, O], f32)

            # Warm-up: the PE HAM clock gate needs ~3.4us of sustained matmul
            # activity to unthrottle 1.2 -> 2.4 GHz. The PE is otherwise idle
            # while the first input chunks DMA in, so burn that window with
            # short matmuls on a zeroed tile; the real matmuls then start at
            # full clock. Keep the total under the DMA head so they never
            # delay real work (PE executes its queue in program order).
            zk = cpool.tile([P, 2, 64], mm_dt)
            nc.gpsimd.memset(zk[:], 0.0)
            ps_warm = ppool.tile([P, NTILE], f32, tag="ps")
            for _ in range(40):
                if FP8:
                    nc.tensor.matmul(
                        ps_warm[:64, :64],
                        lhsT=zk[:],
                        rhs=zk[:],
                        start=True,
                        stop=True,
                        perf_mode=mybir.MatmulPerfMode.DoubleRow,
                    )
                else:
                    nc.tensor.matmul(
                        ps_warm[:64, :64],
                        lhsT=zk[:, 0, :],
                        rhs=zk[:, 0, :],
                        start=True,
                        stop=True,
                    )

            for kc in range(kiters):
                if FP8:
                    nc.sync.dma_start(out=xk_sb[:, kc, :, :], in_=xk_d[:, kc])
                    nc.sync.dma_start(out=wk_sb[:, 0, kc, :, :],
                                      in_=wk_d[0, :, kc])
                else:
                    nc.sync.dma_start(out=xk_sb[:, kc, :], in_=xk_d[:, kc, :])
                    nc.sync.dma_start(out=wk_sb[:, 0, kc, :],
                                      in_=wk_d[0, :, kc, :])
            nc.sync.dma_start(out=xsq_sb[:], in_=xsq_d[:])
            nc.sync.dma_start(out=wk_sb[:, 1], in_=wk_d[1])
            nc.sync.dma_start(out=vb_sb[:], in_=v_d[:].to_broadcast([P, O]))
            nc.sync.dma_start(out=wk_sb[:, 2], in_=wk_d[2])

            # Per n-tile, process the 8 m-tiles as two half-passes of 4 PSUM
            # banks: the PE accumulates into one half while the ACT/DVE
            # eviction chain drains the other (serial ACT frees banks at
            # ~0.7us/bank, slower than the PE's first-kc-pass consumption, so
            # a full 8-bank rotation stalls the PE at every nt boundary).
            for nt in range(NT):
                if nt + 3 < NT:
                    nc.sync.dma_start(out=wk_sb[:, nt + 3], in_=wk_d[nt + 3])
                ns = slice(nt * NTILE, (nt + 1) * NTILE)
                for half in range(2):
                    mts = range(half * (MT // 2), (half + 1) * (MT // 2))
                    pss = {}
                    for mt in mts:
                        ps = ppool.tile([P, NTILE], f32, tag="ps")
                        pss[mt] = ps
                    for kc in range(kiters):
                        for mt in mts:
                            if FP8:
                                nc.tensor.matmul(
                                    pss[mt][:],
                                    lhsT=xk_sb[:, kc, :, mt * P:(mt + 1) * P],
                                    rhs=wk_sb[:, nt, kc, :, :],
                                    start=(kc == 0),
                                    stop=(kc == kiters - 1),
                                    perf_mode=mybir.MatmulPerfMode.DoubleRow,
                                )
                            else:
                                nc.tensor.matmul(
                                    pss[mt][:],
                                    lhsT=xk_sb[:, kc, mt * P:(mt + 1) * P],
                                    rhs=wk_sb[:, nt, kc, :],
                                    start=(kc == 0),
                                    stop=(kc == kiters - 1),
                                )
                    ob = opool.tile([P, MT // 2, NTILE], f32)
                    for j, mt in enumerate(mts):
                        nc.scalar.activation(
                            ob[:, j, :],
                            pss[mt][:],
                            mybir.ActivationFunctionType.Identity,
                            bias=xsq_sb[:, mt:mt + 1],
                            scale=act_scale,
                        )
                        nc.vector.tensor_add(ob[:, j, :], ob[:, j, :], vb_sb[:, ns])
                    mt0 = half * (MT // 2)
                    nc.sync.dma_start(out=out_d[:, mt0:mt0 + MT // 2, ns], in_=ob[:])

    nc.finalize()
    _CACHE[key] = nc
    return nc


def _prep_inputs(x, weights, bias):
    """Shard + lay out host inputs -> per-core in_maps."""
    x = np.asarray(x, dtype=np.float32)
    weights = np.asarray(weights, dtype=np.float32)
    bias = np.asarray(bias, dtype=np.float32)

    w_sq = np.einsum("od,od->o", weights, weights)
    v = np.ascontiguousarray((bias - w_sq / np.float32(D)).reshape(1, O))

    if FP8:
        dt = ml_dtypes.float8_e4m3
        # k = kd*256 + i*128 + p
        wT = weights.T * np.float32(WSCALE)                   # [D, O]
        wk = np.ascontiguousarray(
            wT.reshape(KD, 2, P, NT, NTILE)
            .transpose(3, 2, 0, 1, 4)
            .astype(dt)
        )
    else:
        dt = ml_dtypes.bfloat16
        wT = weights.T * np.float32(2.0 / D)
        wk = np.ascontiguousarray(
            wT.reshape(KT, P, NT, NTILE).transpose(2, 1, 0, 3).astype(dt)
        )

    in_maps = []
    for c in range(NCORES):
        xs = x[c * BL:(c + 1) * BL]                            # [BL, D] fp32
        xT = xs.T                                              # [D, BL]
        if FP8:
            xk = np.ascontiguousarray(
                (xT.reshape(KD, 2, P, BL) * np.float32(XSCALE))
                .transpose(2, 0, 1, 3)
                .astype(dt)
            )
        else:
            xk = np.ascontiguousarray(
                xT.reshape(KT, P, BL).transpose(1, 0, 2).astype(dt)
            )
        xsq = -np.einsum("bd,bd->b", xs, xs) / np.float32(D)   # [BL]
        xsq_l = np.ascontiguousarray(xsq.reshape(MT, P).T)     # [P, MT]
        in_maps.append({"xk": xk, "wk": wk, "xsq": xsq_l, "v": v})
    return in_maps


def _gather(results):
    parts = []
    for c in range(NCORES):
        o = results[c]["out"]                                  # [P, MT, O]
        parts.append(o.transpose(1, 0, 2).reshape(BL, O))
    return np.ascontiguousarray(np.concatenate(parts, axis=0))


def _run(in_maps, **kwargs):
    from concourse.bass_utils import run_bass_kernel_spmd

    nc = _get_nc()
    return run_bass_kernel_spmd(nc, in_maps, core_ids=list(range(NCORES)), **kwargs)


def kernel(x, weights, bias):
    in_maps = _prep_inputs(x, weights, bias)
    res = _run(in_maps)
    return _gather(res.results)


# revision 12
# speedup vs baseline: 1.9349x; 1.0477x over previous
"""Trainium2 Bass kernel for nn_MemLayer (retrieval_knn).

Math:  out[b,o] = -mean_d (x[b,d] - w[o,d])^2 + bias[o]
              =  s * (x' @ w'.T)[b,o]  -  ||x_b||^2/D  +  (bias[o] - ||w_o||^2/D)

  with x' = 16*x, w' = 4096*w in fp8e4m3 and s = 2/(D*16*4096) applied on the
  ACT engine at PSUM eviction (both scale factors keep the fp8 operands inside
  the e4m3 normal range; accumulation is fp32 in PSUM).

Strategy:
  - Data-parallel shard x along batch across 8 NeuronCores (1024 rows each),
    replicate weights. No cross-core communication; gather outputs on host.
  - Per core: fp8 GEMM [1024,1024] @ [1024,4096] using DoubleRow perf mode
    (2 fp8 weights per PE cell -> contraction 256 per matmul, 256 matmuls).
  - Schedule: n-tile outer; within an n-tile the contraction (kd) loop is
    OUTER across all 8 PSUM banks (one per m-tile), so the first matmuls only
    need a few hundred KB of DMA before the PE starts. Weight n-chunks are
    prefetched just-in-time from inside the nt loop so they never sit ahead
    of output evictions in the shared HWDGE FIFOs.
  - Corrections stay fp32, fused into PSUM eviction:
      * ACT:  out_sb = psum * s + xsq[p]   (per-partition bias, -||x||^2/D)
      * DVE:  out_sb += v[o]               (v = bias - ||w||^2/D, row bcast)
    then a 256KB DMA per (m,n) tile straight to DRAM.

The rank-1 reductions (x_sq, w_sq) are computed on the host in fp32, so the
only reduced-precision term is the (2/D)*x.w correction, which is ~1e-3 of
the output scale; elementwise output error stays ~3e-5 relative.
"""

import numpy as np
import ml_dtypes

B, D, O = 8192, 1024, 4096
NCORES = 8
BL = B // NCORES     # 1024 rows per core
P = 128
MT = BL // P         # 8 m-tiles
NTILE = 512          # one PSUM bank of fp32
NT = O // NTILE      # 8 n-tiles

FP8 = True
KT = D // P          # 8 k-tiles (bf16 path)
KD = D // (2 * P)    # 4 double-k-tiles (fp8 DoubleRow path)
XSCALE = 16.0        # x -> fp8 pre-scale
WSCALE = 4096.0      # w -> fp8 pre-scale

_CACHE = {}


def _get_nc():
    key = ("nc", FP8)
    if key in _CACHE:
        return _CACHE[key]

    import concourse.bacc as bacc
    import concourse.tile as tile
    from concourse import mybir

    nc = bacc.Bacc("TRN2", target_bir_lowering=False)

    f32 = mybir.dt.float32
    mm_dt = mybir.dt.float8e4 if FP8 else mybir.dt.bfloat16

    if FP8:
        xk_d = nc.dram_tensor("xk", [P, KD, 2, BL], mm_dt, kind="ExternalInput")
        wk_d = nc.dram_tensor("wk", [NT, P, KD, 2, NTILE], mm_dt,
                              kind="ExternalInput")
    else:
        xk_d = nc.dram_tensor("xk", [P, KT, BL], mm_dt, kind="ExternalInput")
        wk_d = nc.dram_tensor("wk", [NT, P, KT, NTILE], mm_dt,
                              kind="ExternalInput")
    xsq_d = nc.dram_tensor("xsq", [P, MT], f32, kind="ExternalInput")
    v_d = nc.dram_tensor("v", [1, O], f32, kind="ExternalInput")
    out_d = nc.dram_tensor("out", [P, MT, O], f32, kind="ExternalOutput")

    act_scale = float(2.0 / (D * XSCALE * WSCALE)) if FP8 else 1.0
    kiters = KD if FP8 else KT

    with tile.TileContext(nc) as tc:
        with (
            tc.tile_pool(name="const", bufs=1) as cpool,
            tc.tile_pool(name="psum", bufs=8, space="PSUM") as ppool,
            tc.tile_pool(name="outp", bufs=4) as opool,
        ):
            if FP8:
                xk_sb = cpool.tile([P, KD, 2, BL], mm_dt)
                wk_sb = cpool.tile([P, NT, KD, 2, NTILE], mm_dt)
            else:
                xk_sb = cpool.tile([P, KT, BL], mm_dt)
                wk_sb = cpool.tile([P, NT, KT, NTILE], mm_dt)
            xsq_sb = cpool.tile([P, MT], f32)
            vb_sb = cpool.tile([P, O], f32)

            # Warm-up: the PE HAM clock gate needs ~3.4us of sustained matmul
            # activity to unthrottle 1.2 -> 2.4 GHz. The PE is otherwise idle
            # while the first input chunks DMA in, so burn that window with
            # short matmuls on a zeroed tile; the real matmuls then start at
            # full clock. Keep the total under the DMA head so they never
            # delay real work (PE executes its queue in program order).
            zk = cpool.tile([P, 2, 64], mm_dt)
            nc.gpsimd.memset(zk[:], 0.0)
            ps_warm = ppool.tile([P, NTILE], f32, tag="ps")
            for _ in range(60):
                if FP8:
                    nc.tensor.matmul(
                        ps_warm[:64, :64],
                        lhsT=zk[:],
                        rhs=zk[:],
                        start=True,
                        stop=True,
                        perf_mode=mybir.MatmulPerfMode.DoubleRow,
                    )
                else:
                    nc.tensor.matmul(
                        ps_warm[:64, :64],
                        lhsT=zk[:, 0, :],
                        rhs=zk[:, 0, :],
                        start=True,
                        stop=True,
                    )

            # xk chunks enqueue on the Activation engine's DGE rings so they
            # don't serialize behind the Sync-issued weight chunks (~600ns
            # enqueue each); both streams start in parallel at t~7us.
            for kc in range(kiters):
                if FP8:
                    nc.scalar.dma_start(out=xk_sb[:, kc, :, :], in_=xk_d[:, kc])
                    nc.sync.dma_start(out=wk_sb[:, 0, kc, :, :],
                                      in_=wk_d[0, :, kc])
                else:
                    nc.scalar.dma_start(out=xk_sb[:, kc, :], in_=xk_d[:, kc, :])
                    nc.sync.dma_start(out=wk_sb[:, 0, kc, :],
                                      in_=wk_d[0, :, kc, :])
            nc.sync.dma_start(out=xsq_sb[:], in_=xsq_d[:])
            nc.sync.dma_start(out=wk_sb[:, 1], in_=wk_d[1])
            nc.sync.dma_start(out=vb_sb[:], in_=v_d[:].to_broadcast([P, O]))
            nc.sync.dma_start(out=wk_sb[:, 2], in_=wk_d[2])

            # Per n-tile, process the 8 m-tiles as two half-passes of 4 PSUM
            # banks: the PE accumulates into one half while the ACT/DVE
            # eviction chain drains the other (serial ACT frees banks at
            # ~0.7us/bank, slower than the PE's first-kc-pass consumption, so
            # a full 8-bank rotation stalls the PE at every nt boundary).
            for nt in range(NT):
                if nt + 3 < NT:
                    nc.sync.dma_start(out=wk_sb[:, nt + 3], in_=wk_d[nt + 3])
                ns = slice(nt * NTILE, (nt + 1) * NTILE)
                for half in range(2):
                    mts = range(half * (MT // 2), (half + 1) * (MT // 2))
                    pss = {}
                    for mt in mts:
                        ps = ppool.tile([P, NTILE], f32, tag="ps")
                        pss[mt] = ps
                    for kc in range(kiters):
                        for mt in mts:
                            if FP8:
                                nc.tensor.matmul(
                                    pss[mt][:],
                                    lhsT=xk_sb[:, kc, :, mt * P:(mt + 1) * P],
                                    rhs=wk_sb[:, nt, kc, :, :],
                                    start=(kc == 0),
                                    stop=(kc == kiters - 1),
                                    perf_mode=mybir.MatmulPerfMode.DoubleRow,
                                )
                            else:
                                nc.tensor.matmul(
                                    pss[mt][:],
                                    lhsT=xk_sb[:, kc, mt * P:(mt + 1) * P],
                                    rhs=wk_sb[:, nt, kc, :],
                                    start=(kc == 0),
                                    stop=(kc == kiters - 1),
                                )
                    if nt == NT - 1 and half == 1:
                        # Final half: per-tile eviction DMAs so the kernel
                        # tail is one small chain instead of a batched 1MB
                        # transfer gated on all four DVE adds.
                        for mt in mts:
                            obs = opool.tile([P, NTILE], f32, tag="obs")
                            nc.scalar.activation(
                                obs[:],
                                pss[mt][:],
                                mybir.ActivationFunctionType.Identity,
                                bias=xsq_sb[:, mt:mt + 1],
                                scale=act_scale,
                            )
                            nc.vector.tensor_add(obs[:], obs[:], vb_sb[:, ns])
                            nc.sync.dma_start(out=out_d[:, mt, ns], in_=obs[:])
                    else:
                        ob = opool.tile([P, MT // 2, NTILE], f32)
                        for j, mt in enumerate(mts):
                            nc.scalar.activation(
                                ob[:, j, :],
                                pss[mt][:],
                                mybir.ActivationFunctionType.Identity,
                                bias=xsq_sb[:, mt:mt + 1],
                                scale=act_scale,
                            )
                            nc.vector.tensor_add(ob[:, j, :], ob[:, j, :], vb_sb[:, ns])
                        mt0 = half * (MT // 2)
                        nc.sync.dma_start(out=out_d[:, mt0:mt0 + MT // 2, ns], in_=ob[:])

    nc.finalize()
    _CACHE[key] = nc
    return nc


def _prep_inputs(x, weights, bias):
    """Shard + lay out host inputs -> per-core in_maps."""
    x = np.asarray(x, dtype=np.float32)
    weights = np.asarray(weights, dtype=np.float32)
    bias = np.asarray(bias, dtype=np.float32)

    w_sq = np.einsum("od,od->o", weights, weights)
    v = np.ascontiguousarray((bias - w_sq / np.float32(D)).reshape(1, O))

    if FP8:
        dt = ml_dtypes.float8_e4m3
        # k = kd*256 + i*128 + p
        wT = weights.T * np.float32(WSCALE)                   # [D, O]
        wk = np.ascontiguousarray(
            wT.reshape(KD, 2, P, NT, NTILE)
            .transpose(3, 2, 0, 1, 4)
            .astype(dt)
        )
    else:
        dt = ml_dtypes.bfloat16
        wT = weights.T * np.float32(2.0 / D)
        wk = np.ascontiguousarray(
            wT.reshape(KT, P, NT, NTILE).transpose(2, 1, 0, 3).astype(dt)
        )

    in_maps = []
    for c in range(NCORES):
        xs = x[c * BL:(c + 1) * BL]                            # [BL, D] fp32
        xT = xs.T                                              # [D, BL]
        if FP8:
            xk = np.ascontiguousarray(
                (xT.reshape(KD, 2, P, BL) * np.float32(XSCALE))
                .transpose(2, 0, 1, 3)
                .astype(dt)
            )
        else:
            xk = np.ascontiguousarray(
                xT.reshape(KT, P, BL).transpose(1, 0, 2).astype(dt)
            )
        xsq = -np.einsum("bd,bd->b", xs, xs) / np.float32(D)   # [BL]
        xsq_l = np.ascontiguousarray(xsq.reshape(MT, P).T)     # [P, MT]
        in_maps.append({"xk": xk, "wk": wk, "xsq": xsq_l, "v": v})
    return in_maps


def _gather(results):
    parts = []
    for c in range(NCORES):
        o = results[c]["out"]                                  # [P, MT, O]
        parts.append(o.transpose(1, 0, 2).reshape(BL, O))
    return np.ascontiguousarray(np.concatenate(parts, axis=0))


def _run(in_maps, **kwargs):
    from concourse.bass_utils import run_bass_kernel_spmd

    nc = _get_nc()
    return run_bass_kernel_spmd(nc, in_maps, core_ids=list(range(NCORES)), **kwargs)


def kernel(x, weights, bias):
    in_maps = _prep_inputs(x, weights, bias)
    res = _run(in_maps)
    return _gather(res.results)
